# revision 1
# baseline (speedup 1.0000x reference)
"""DeepseekMoE layer on 8 Trainium2 NeuronCores (Bass/Tile, expert-parallel).

Sharding (per the expert-parallel hint):
  - 16 routed experts -> 2 per core; token dispatch (all-to-all) is emulated at
    the sharding layer: the host computes the discrete top-4 routing, gathers
    each expert's tokens into a compact transposed batch, and scatter-adds the
    compact expert outputs back into the full output ("combine").
  - Shared expert is tensor-parallel over its intermediate dim (2816/8 = 352
    columns per core); the 8 partial outputs are summed on gather.
  - Gate (softmax + renormalized top-4 combine weights) is replicated and
    computed ON DEVICE from the hidden states; the host only supplies the
    discrete 0/1 top-4 mask (routing decision) and gather indices.

All FLOPs that produce output values run on device. Matmuls use float32r
(full-rate fp32 mode, ~1.5e-4 rel-rms) except the tiny gate matmul which uses
exact 2-pass fp32.

Weights are host-packed into stationary-tile-major layout ([m-tile, partition,
k-tile, col]) so each m-column's whole contraction loads as one DMA with
multi-KB descriptors: the Sync engine costs ~620ns per DMA instruction
regardless of size, so small-tile DMA streams are issue-bound.
"""

import os
import numpy as np

H = 2048          # hidden size
E = 16            # routed experts
TOPK = 4
I = 1408          # routed expert intermediate
ISH = 2816        # shared expert intermediate
T = 1024          # tokens
P = 128
NCORES = 8
EPC = 2           # experts per core
ISS = ISH // NCORES                  # 352 shared columns per core
ISSP = 384                           # padded to 3 full 128-tiles
KH = H // P                          # 16 k-tiles over H
MI = I // P                          # 11 m-tiles over I
MH = H // P                          # 16 m-tiles over H
KI = I // P                          # 11 k-tiles over I
KS = ISSP // P                       # 3 k-tiles over padded shared slice
ZERO_ROW_FLAT = T * E                # flat index of the zeroed scratch row

_NC_CACHE = {}
LAST_RESULTS = None  # BassKernelResults of the most recent run (for test.py)


def _token_chunks(C):
    """Split [0, C) into matmul moving-dim chunks of <=512."""
    out = []
    off = 0
    while off < C:
        sz = min(512, C - off)
        out.append((off, sz))
        off += sz
    return out


def _pack_st(w, KT, MT):
    """[KT*P, MT*P] -> [MT*P, KT*P] tile-major stationary pack.

    packed[m*P + p, k*P + c] = w[k*P + p, m*P + c], so the device loads
    rows [m*P, (m+1)*P) as one [P, KT*P] block whose column-slice k is the
    stationary tile for (k, m).
    """
    return np.ascontiguousarray(
        w.reshape(KT, P, MT, P).transpose(2, 1, 0, 3).reshape(MT * P, KT * P))


def _build(C):
    import concourse.bacc as bacc
    import concourse.bass as bass
    import concourse.mybir as mybir
    import concourse.tile as tile
    from concourse.masks import make_identity

    f32 = mybir.dt.float32
    f32r = mybir.dt.float32r
    i32 = mybir.dt.int32
    SILU = mybir.ActivationFunctionType.Silu
    EXP = mybir.ActivationFunctionType.Exp
    X = mybir.AxisListType.X

    CH = _token_chunks(C)
    NT = T // 512     # token chunks for shared/gate (2)

    nc = bacc.Bacc("TRN2", target_bir_lowering=False, debug=False)

    xt_h = nc.dram_tensor("xt", [H, T], f32r, kind="ExternalInput")
    gwtb_h = nc.dram_tensor("gwtb", [P, KH * E], f32r, kind="ExternalInput")
    maskb_h = nc.dram_tensor("maskb", [P, (T // P) * E], f32, kind="ExternalInput")
    xg_h = [nc.dram_tensor(f"xg{j}", [P, KH * C], f32r, kind="ExternalInput") for j in range(EPC)]
    widx_h = nc.dram_tensor("widx", [EPC * C, 1], i32, kind="ExternalInput")
    wg_h = [nc.dram_tensor(f"wg{j}", [I, H], f32r, kind="ExternalInput") for j in range(EPC)]
    wu_h = [nc.dram_tensor(f"wu{j}", [I, H], f32r, kind="ExternalInput") for j in range(EPC)]
    wd_h = [nc.dram_tensor(f"wd{j}", [H, I], f32r, kind="ExternalInput") for j in range(EPC)]
    swg_h = nc.dram_tensor("swg", [ISSP, H], f32r, kind="ExternalInput")
    swu_h = nc.dram_tensor("swu", [ISSP, H], f32r, kind="ExternalInput")
    swd_h = nc.dram_tensor("swd", [H, ISSP], f32r, kind="ExternalInput")
    zt_h = nc.dram_tensor("zt", [H, EPC * C], f32, kind="ExternalOutput")
    st_h = nc.dram_tensor("st", [H, T], f32, kind="ExternalOutput")

    with tile.TileContext(nc) as tc:
        with (
            tc.tile_pool(name="resident", bufs=1) as res_pool,
            tc.tile_pool(name="xgp", bufs=1) as xg_pool,
            tc.tile_pool(name="acts", bufs=1) as act_pool,
            tc.tile_pool(name="wstream", bufs=3) as wst_pool,
            tc.tile_pool(name="dstream", bufs=3) as dst_pool,
            tc.tile_pool(name="sstream", bufs=2) as sst_pool,
            tc.tile_pool(name="small", bufs=2) as small_pool,
            tc.tile_pool(name="stage", bufs=3) as stage_pool,
            tc.tile_pool(name="ps", bufs=1, space="PSUM") as ps_pool,
            tc.tile_pool(name="dram", bufs=1, space="DRAM") as dram_pool,
        ):
            # ---------------- resident loads ----------------
            # expert-0 activations first so routed matmuls start before the
            # (large) xt load completes
            xgb = [xg_pool.tile([P, KH * C], f32r, name=f"xgb{j}", tag="xgb") for j in range(EPC)]
            nc.sync.dma_start(xgb[0][:], xg_h[0][:])
            xt2 = [res_pool.tile([P, 2 * T], f32r, name=f"xt2_{kk}", tag=f"xt2_{kk}") for kk in range(KH // 2)]
            for kk in range(KH // 2):
                nc.sync.dma_start(
                    xt2[kk][:].rearrange("p (a t) -> p a t", a=2),
                    xt_h[kk * 2 * P:(kk + 1) * 2 * P, :].rearrange("(a p) t -> p a t", p=P))
            xt_t = [xt2[k // 2][:, (k % 2) * T:(k % 2 + 1) * T] for k in range(KH)]
            nc.sync.dma_start(xgb[1][:], xg_h[1][:])
            gwtb = res_pool.tile([P, KH * E], f32r, name="gwtb", tag="gwtb")
            nc.gpsimd.dma_start(gwtb[:], gwtb_h[:])
            maskb = res_pool.tile([P, (T // P) * E], f32, name="maskb", tag="maskb")
            nc.gpsimd.dma_start(maskb[:], maskb_h[:])
            ident = res_pool.tile([P, P], f32, name="ident", tag="ident")
            make_identity(nc, ident[:])
            zbias = res_pool.tile([P, 1], f32, name="zbias", tag="zbias")
            nc.vector.memset(zbias[:], 0.0)

            # combine-weight scratch in HBM: rows 0..T-1 = combine, row T = zeros
            wflat = dram_pool.tile([(T + 1) * E, 1], f32, name="wflat")
            wflat2d = wflat[:].rearrange("(a b) o -> a (b o)", b=E)
            zrow = res_pool.tile([1, E], f32, name="zrow", tag="zrow")
            nc.vector.memset(zrow[:], 0.0)
            nc.gpsimd.dma_start(wflat2d[T:T + 1, :], zrow[:])

            wb = [res_pool.tile([P, C], f32, name=f"wb{j}", tag=f"wb{j}") for j in range(EPC)]
            a_t = [[act_pool.tile([P, C], f32r, name=f"a{j}_{m}", tag=f"a{j}_{m}") for m in range(MI)]
                   for j in range(EPC)]
            sg_t = [act_pool.tile([P, T], f32, name=f"sg{m}", tag="sgtmp", bufs=2) for m in range(KS)]
            as_t = [act_pool.tile([P, T], f32r, name=f"as{m}", tag=f"as{m}") for m in range(KS)]

            # ---------------- emission sections ----------------
            def emit_gate():
                lgps = ps_pool.tile([E, T], f32, name="lgps", tag="B1", bufs=2)
                for n in range(NT):
                    for k in range(KH):
                        nc.tensor.matmul(
                            lgps[:, n * 512:(n + 1) * 512],
                            lhsT=gwtb[:, k * E:(k + 1) * E],
                            rhs=xt_t[k][:, n * 512:(n + 1) * 512],
                            start=(k == 0), stop=(k == KH - 1),
                        )
                lgsb = res_pool.tile([E, T], f32, name="lgsb", tag="lgsb")
                nc.scalar.copy(lgsb[:], lgps[:])
                for t8 in range(T // P):
                    trps = ps_pool.tile([P, E], f32, name=f"tr{t8}", tag="A1", bufs=4)
                    nc.tensor.transpose(
                        out=trps[:], in_=lgsb[:, t8 * P:(t8 + 1) * P], identity=ident[0:E, 0:E],
                    )
                    sc = small_pool.tile([P, E], f32, name=f"sc{t8}", tag="sc")
                    nc.scalar.activation(sc[:], trps[:], EXP, bias=zbias[:])
                    mskd = small_pool.tile([P, E], f32, name=f"mskd{t8}", tag="mskd")
                    nc.vector.tensor_mul(out=mskd[:], in0=sc[:], in1=maskb[:, t8 * E:(t8 + 1) * E])
                    ssum = small_pool.tile([P, 1], f32, name=f"ssum{t8}", tag="ssum")
                    nc.vector.reduce_sum(ssum[:], mskd[:], axis=X)
                    rsum = small_pool.tile([P, 1], f32, name=f"rsum{t8}", tag="rsum")
                    nc.vector.reciprocal(rsum[:], ssum[:])
                    comb = small_pool.tile([P, E], f32, name=f"comb{t8}", tag="comb")
                    nc.vector.tensor_scalar_mul(comb[:], mskd[:], rsum[:, :1])
                    nc.gpsimd.dma_start(wflat2d[t8 * P:(t8 + 1) * P, :], comb[:])

            def emit_gather(j):
                # per-slot combine weights -> partition-broadcast wb[j]
                off = 0
                while off < C:
                    csz = min(P, C - off)
                    it = small_pool.tile([P, 1], i32, name=f"it{j}_{off}", tag="it")
                    nc.gpsimd.dma_start(it[:csz], widx_h[j * C + off:j * C + off + csz, :])
                    wslot = small_pool.tile([P, 1], f32, name=f"ws{j}_{off}", tag="ws")
                    nc.gpsimd.indirect_dma_start(
                        out=wslot[:csz, :], out_offset=None, in_=wflat[:],
                        in_offset=bass.IndirectOffsetOnAxis(ap=it[:csz, :1], axis=0),
                    )
                    wbps = ps_pool.tile([P, P], f32, name=f"wbps{j}_{off}", tag="A1", bufs=4)
                    nc.tensor.transpose(
                        out=wbps[:, :csz],
                        in_=wslot[:csz, :1].to_broadcast([csz, P]),
                        identity=ident[0:csz, 0:csz],
                    )
                    nc.vector.tensor_copy(wb[j][:, off:off + csz], wbps[:, :csz])
                    off += csz

            def emit_upgate(j):
                xg_t = [xgb[j][:, k * C:(k + 1) * C] for k in range(KH)]
                g_t = [act_pool.tile([P, C], f32, name=f"g{j}_{m}", tag="gtmp", bufs=3) for m in range(MI)]
                for m in range(MI):
                    wgb1 = wst_pool.tile([P, KH * P], f32r, name=f"wgb{j}_{m}", tag="wblk", bufs=3)
                    nc.sync.dma_start(wgb1[:], wg_h[j][m * P:(m + 1) * P, :])
                    wub1 = wst_pool.tile([P, KH * P], f32r, name=f"wub{j}_{m}", tag="wblk", bufs=3)
                    nc.sync.dma_start(wub1[:], wu_h[j][m * P:(m + 1) * P, :])
                    wgb = [wgb1[:, :8 * P], wgb1[:, 8 * P:]]
                    wub = [wub1[:, :8 * P], wub1[:, 8 * P:]]
                    for (coff, csz) in CH:
                        psg = ps_pool.tile([P, csz], f32, name=f"psg{j}_{m}_{coff}", tag="A1", bufs=4)
                        for k in range(KH):
                            nc.tensor.matmul(psg[:], lhsT=wgb[k // 8][:, (k % 8) * P:(k % 8 + 1) * P],
                                             rhs=xg_t[k][:, coff:coff + csz],
                                             start=(k == 0), stop=(k == KH - 1))
                        nc.scalar.activation(g_t[m][:, coff:coff + csz], psg[:], SILU, bias=zbias[:])
                        psu = ps_pool.tile([P, csz], f32, name=f"psu{j}_{m}_{coff}", tag="A1", bufs=4)
                        for k in range(KH):
                            nc.tensor.matmul(psu[:], lhsT=wub[k // 8][:, (k % 8) * P:(k % 8 + 1) * P],
                                             rhs=xg_t[k][:, coff:coff + csz],
                                             start=(k == 0), stop=(k == KH - 1))
                        # a = silu(g) * u straight out of PSUM, rounded to f32r
                        nc.vector.tensor_mul(out=a_t[j][m][:, coff:coff + csz],
                                             in0=g_t[m][:, coff:coff + csz], in1=psu[:])

            def emit_down(j, interleave=None):
                for m in range(MH):
                    if interleave is not None and m % 2 == 0:
                        interleave(m // 2)
                    wdb1 = dst_pool.tile([P, KI * P], f32r, name=f"wdb{j}_{m}", tag="wdb", bufs=2)
                    nc.sync.dma_start(wdb1[:], wd_h[j][m * P:(m + 1) * P, :])
                    wdb = [wdb1[:, :6 * P], wdb1[:, 6 * P:]]
                    for (coff, csz) in CH:
                        psz = ps_pool.tile([P, csz], f32, name=f"psz{j}_{m}_{coff}", tag="A1", bufs=4)
                        for k in range(KI):
                            nc.tensor.matmul(psz[:], lhsT=wdb1[:, k * P:(k + 1) * P],
                                             rhs=a_t[j][k][:, coff:coff + csz],
                                             start=(k == 0), stop=(k == KI - 1))
                        zst = stage_pool.tile([P, csz], f32, name=f"zst{j}_{m}_{coff}", tag="zst", bufs=2)
                        # combine-weight scaling fused into the eviction
                        nc.vector.tensor_mul(out=zst[:], in0=wb[j][:, coff:coff + csz], in1=psz[:])
                        nc.gpsimd.dma_start(zt_h[m * P:(m + 1) * P, j * C + coff:j * C + coff + csz], zst[:])

            def emit_shared_ug(mi):
                sgb1 = sst_pool.tile([P, KH * P], f32r, name=f"sgb{mi}", tag="ssb", bufs=2)
                nc.sync.dma_start(sgb1[:], swg_h[mi * P:(mi + 1) * P, :])
                sgb = [sgb1[:, :8 * P], sgb1[:, 8 * P:]]
                psgs = ps_pool.tile([P, T], f32, name=f"psgs{mi}", tag="B1", bufs=2)
                for k in range(KH):
                    for n in range(NT):
                        nc.tensor.matmul(psgs[:, n * 512:(n + 1) * 512],
                                         lhsT=sgb[k // 8][:, (k % 8) * P:(k % 8 + 1) * P],
                                         rhs=xt_t[k][:, n * 512:(n + 1) * 512],
                                         start=(k == 0), stop=(k == KH - 1))
                nc.scalar.activation(sg_t[mi][:], psgs[:], SILU, bias=zbias[:])
                sub1 = sst_pool.tile([P, KH * P], f32r, name=f"sub{mi}", tag="ssb", bufs=2)
                nc.sync.dma_start(sub1[:], swu_h[mi * P:(mi + 1) * P, :])
                sub = [sub1[:, :8 * P], sub1[:, 8 * P:]]
                psus = ps_pool.tile([P, T], f32, name=f"psus{mi}", tag="B1", bufs=2)
                for k in range(KH):
                    for n in range(NT):
                        nc.tensor.matmul(psus[:, n * 512:(n + 1) * 512],
                                         lhsT=sub[k // 8][:, (k % 8) * P:(k % 8 + 1) * P],
                                         rhs=xt_t[k][:, n * 512:(n + 1) * 512],
                                         start=(k == 0), stop=(k == KH - 1))
                nc.vector.tensor_mul(out=as_t[mi][:], in0=sg_t[mi][:], in1=psus[:])

            def emit_shared_down(ms):
                for m in ms:
                    sdb = sst_pool.tile([P, KS * P], f32r, name=f"sdb{m}", tag="sdb", bufs=2)
                    nc.sync.dma_start(sdb[:], swd_h[m * P:(m + 1) * P, :])
                    psys = ps_pool.tile([P, T], f32, name=f"psys{m}", tag="B1", bufs=2)
                    for ki in range(KS):
                        for n in range(NT):
                            nc.tensor.matmul(psys[:, n * 512:(n + 1) * 512],
                                             lhsT=sdb[:, ki * P:(ki + 1) * P],
                                             rhs=as_t[ki][:, n * 512:(n + 1) * 512],
                                             start=(ki == 0), stop=(ki == KS - 1))
                    sstg = stage_pool.tile([P, T], f32, name=f"sstg{m}", tag="sstage", bufs=2)
                    nc.scalar.copy(sstg[:], psys[:])
                    nc.sync.dma_start(st_h[m * P:(m + 1) * P, :], sstg[:])

            emit_gate()
            emit_upgate(0)
            emit_gather(0)
            emit_shared_ug(0)
            emit_down(0)
            emit_shared_ug(1)
            emit_upgate(1)
            emit_gather(1)
            emit_shared_ug(2)
            emit_down(1)
            emit_shared_down(list(range(MH)))

    nc.compile()
    return nc


def _get_nc(C):
    if C not in _NC_CACHE:
        _NC_CACHE[C] = _build(C)
    return _NC_CACHE[C]


def kernel(**inputs):
    global LAST_RESULTS
    from concourse.bass_utils import run_bass_kernel_spmd

    hs = np.asarray(inputs["hidden_states"], dtype=np.float32)
    gate_w = np.asarray(inputs["gate_w"], dtype=np.float32)
    w_gate = np.asarray(inputs["w_gate"], dtype=np.float32)
    w_up = np.asarray(inputs["w_up"], dtype=np.float32)
    w_down = np.asarray(inputs["w_down"], dtype=np.float32)
    sw_gate = np.asarray(inputs["sw_gate"], dtype=np.float32)
    sw_up = np.asarray(inputs["sw_up"], dtype=np.float32)
    sw_down = np.asarray(inputs["sw_down"], dtype=np.float32)

    orig_shape = hs.shape
    x = hs.reshape(-1, H)
    assert x.shape[0] == T

    # ---- host: discrete routing only (top-4 selection + dispatch tables) ----
    logits = x @ gate_w.T
    smax = logits.max(axis=-1, keepdims=True)
    sc = np.exp(logits - smax)
    sc /= sc.sum(axis=-1, keepdims=True)
    order = np.argsort(-sc, axis=-1, kind="stable")[:, :TOPK]
    mask = np.zeros((T, E), dtype=np.float32)
    mask[np.arange(T)[:, None], order] = 1.0
    tok_lists = [np.nonzero(mask[:, e])[0].astype(np.int64) for e in range(E)]
    maxn = max(len(tk) for tk in tok_lists)
    C = max(64, int(np.ceil(maxn / 32)) * 32)

    nc = _get_nc(C)

    xT = np.ascontiguousarray(x.T)
    # gate weights packed: gwtb[p, k*E + e] = gate_w[e, k*P + p]
    gwtb = np.ascontiguousarray(gate_w.T.reshape(KH, P, E).transpose(1, 0, 2).reshape(P, KH * E))
    # mask packed: maskb[p, t8*E + e] = mask[t8*P + p, e]
    maskb = np.ascontiguousarray(mask.reshape(T // P, P, E).transpose(1, 0, 2).reshape(P, (T // P) * E))

    # shared slices, zero-padded to 384 and tile-major packed
    def pad_cols(w, newc):
        out = np.zeros((w.shape[0], newc), dtype=np.float32)
        out[:, :w.shape[1]] = w
        return out

    def pad_rows(w, newr):
        out = np.zeros((newr, w.shape[1]), dtype=np.float32)
        out[:w.shape[0], :] = w
        return out

    in_maps = []
    for c in range(NCORES):
        es = [EPC * c + j for j in range(EPC)]
        widx = np.full((EPC * C, 1), ZERO_ROW_FLAT, dtype=np.int32)
        im = {
            "xt": xT, "gwtb": gwtb, "maskb": maskb, "widx": widx,
            "swg": _pack_st(pad_cols(sw_gate[:, c * ISS:(c + 1) * ISS], ISSP), KH, KS),
            "swu": _pack_st(pad_cols(sw_up[:, c * ISS:(c + 1) * ISS], ISSP), KH, KS),
            "swd": _pack_st(pad_rows(sw_down[c * ISS:(c + 1) * ISS, :], ISSP), KS, MH),
        }
        for j, e in enumerate(es):
            tk = tok_lists[e]
            widx[j * C:j * C + len(tk), 0] = (tk * E + e).astype(np.int32)
            # gathered activations, tile-major: xg[p, k*C + c] = x[tok_c, k*P + p]
            xg = np.zeros((P, KH * C), dtype=np.float32)
            g = xT[:, tk].reshape(KH, P, len(tk)).transpose(1, 0, 2)  # [P, KH, n]
            xg.reshape(P, KH, C)[:, :, :len(tk)] = g
            im[f"xg{j}"] = xg
            im[f"wg{j}"] = _pack_st(w_gate[e], KH, MI)
            im[f"wu{j}"] = _pack_st(w_up[e], KH, MI)
            im[f"wd{j}"] = _pack_st(w_down[e], KI, MH)
        in_maps.append(im)

    trace = bool(int(os.environ.get("BASSMOE_TRACE", "0")))
    kwargs = {}
    if trace:
        kwargs = dict(trace=True, tmpdir=os.environ.get("BASSMOE_TRACE_DIR") or None)
        tcores = os.environ.get("BASSMOE_TRACE_CORES")
        if tcores:
            kwargs["trace_cores"] = [int(x) for x in tcores.split(",")]
            kwargs["stitch_traces"] = False
    res = run_bass_kernel_spmd(nc, in_maps, core_ids=list(range(NCORES)), **kwargs)
    LAST_RESULTS = res

    # ---- host: unshard (scatter-add compact expert outputs + sum partials) ----
    y = np.zeros((T, H), dtype=np.float64)
    st_sum = np.zeros((H, T), dtype=np.float64)
    for c in range(NCORES):
        r = res.results[c]
        st_sum += r["st"]
        for j in range(EPC):
            e = EPC * c + j
            tk = tok_lists[e]
            y[tk] += r["zt"][:, j * C:j * C + len(tk)].T
    y += st_sum.T
    return y.astype(np.float32).reshape(orig_shape)



# revision 2
# speedup vs baseline: 1.4865x; 1.4865x over previous
"""DeepseekMoE layer on 8 Trainium2 NeuronCores (Bass/Tile, expert-parallel).

Sharding (per the expert-parallel hint):
  - 16 routed experts -> 2 per core, paired big+small by token count so the
    per-core slot totals balance; token dispatch (all-to-all) is emulated at
    the sharding layer: the host computes the discrete top-4 routing, gathers
    each expert's tokens into a compact transposed batch, and scatter-adds the
    compact expert outputs back into the full output ("combine").
  - Shared expert is tensor-parallel over its intermediate dim (2816/8 = 352
    columns per core, padded to 384); the 8 partial outputs are summed on
    gather.
  - Gate (softmax + renormalized top-4 combine weights) is replicated and
    computed ON DEVICE from the hidden states; the host only supplies the
    discrete 0/1 top-4 mask (routing decision) and gather indices.

All FLOPs that produce output values run on device.  Matmul operands are
bf16 (fp32 PSUM accumulation): on TRN2 the PE runs bf16 and fp32r at the
same 1 row/cycle, so bf16 costs no compute but halves the HBM weight/
activation traffic, which is what bounds this kernel.  Only the tiny gate
softmax pipeline stays fp32.

Weights are host-packed into stationary-tile-major layout ([m-tile,
partition, k-tile, col]); gate+up (and consecutive down-proj m-tiles) are
further interleaved so one DMA instruction streams 0.7-1MB with 4KB
descriptors, keeping the DMA issue queues short.
"""

import os
import numpy as np

H = 2048          # hidden size
E = 16            # routed experts
TOPK = 4
I = 1408          # routed expert intermediate
ISH = 2816        # shared expert intermediate
T = 1024          # tokens
P = 128
NCORES = 8
EPC = 2           # experts per core
ISS = ISH // NCORES                  # 352 shared columns per core
ISSP = 384                           # padded to 3 full 128-tiles
KH = H // P                          # 16 k-tiles over H
MI = I // P                          # 11 m-tiles over I
MH = H // P                          # 16 m-tiles over H
KI = I // P                          # 11 k-tiles over I
KS = ISSP // P                       # 3 k-tiles over padded shared slice
ZERO_ROW_FLAT = T * E                # flat index of the zeroed scratch row

_NC_CACHE = {}
LAST_RESULTS = None  # BassKernelResults of the most recent run (for test.py)


def _bf16():
    import ml_dtypes
    return ml_dtypes.bfloat16


def _token_chunks(C):
    """Split [0, C) into matmul moving-dim chunks of <=512."""
    out = []
    off = 0
    while off < C:
        sz = min(512, C - off)
        out.append((off, sz))
        off += sz
    return out


def _pack_st(w, KT, MT):
    """[KT*P, MT*P] -> [MT*P, KT*P] tile-major stationary pack.

    packed[m*P + p, k*P + c] = w[k*P + p, m*P + c], so the device loads
    rows [m*P, (m+1)*P) as one [P, KT*P] block whose column-slice k is the
    stationary tile for (k, m).
    """
    return np.ascontiguousarray(
        w.reshape(KT, P, MT, P).transpose(2, 1, 0, 3).reshape(MT * P, KT * P))


def _interleave_pairs(a, b, MT, KTP):
    """Two [MT*P, KTP] packs -> [2*MT*P, KTP] with row-blocks (m,0)=a_m, (m,1)=b_m."""
    return np.ascontiguousarray(
        np.stack([a.reshape(MT, P, KTP), b.reshape(MT, P, KTP)], axis=1)
        .reshape(2 * MT * P, KTP))


def _build(CA, CB):
    import concourse.bacc as bacc
    import concourse.bass as bass
    import concourse.mybir as mybir
    import concourse.tile as tile
    from concourse.masks import make_identity

    f32 = mybir.dt.float32
    bf16 = mybir.dt.bfloat16
    i32 = mybir.dt.int32
    SILU = mybir.ActivationFunctionType.Silu
    EXP = mybir.ActivationFunctionType.Exp
    X = mybir.AxisListType.X

    CJ = [CA, CB]
    CHJ = [_token_chunks(CA), _token_chunks(CB)]
    COFFJ = [0, CA]        # column offsets in widx/zt
    CT = CA + CB
    NT = T // 512     # token chunks for shared/gate (2)

    nc = bacc.Bacc("TRN2", target_bir_lowering=False, debug=False)

    xt_h = nc.dram_tensor("xt", [H, T], bf16, kind="ExternalInput")
    gwtb_h = nc.dram_tensor("gwtb", [P, KH * E], bf16, kind="ExternalInput")
    maskb_h = nc.dram_tensor("maskb", [P, (T // P) * E], f32, kind="ExternalInput")
    xg_h = [nc.dram_tensor(f"xg{j}", [P, KH * CJ[j]], bf16, kind="ExternalInput") for j in range(EPC)]
    widx_h = nc.dram_tensor("widx", [CT, 1], i32, kind="ExternalInput")
    wgu_h = [nc.dram_tensor(f"wgu{j}", [2 * MI * P, KH * P], bf16, kind="ExternalInput") for j in range(EPC)]
    wd_h = [nc.dram_tensor(f"wd{j}", [MH * P, KI * P], bf16, kind="ExternalInput") for j in range(EPC)]
    swgu_h = nc.dram_tensor("swgu", [2 * KS * P, KH * P], bf16, kind="ExternalInput")
    swd_h = nc.dram_tensor("swd", [MH * P, KS * P], bf16, kind="ExternalInput")
    zt_h = nc.dram_tensor("zt", [H, CT], bf16, kind="ExternalOutput")
    st_h = nc.dram_tensor("st", [H, T], bf16, kind="ExternalOutput")

    with tile.TileContext(nc) as tc:
        with (
            tc.tile_pool(name="resident", bufs=1) as res_pool,
            tc.tile_pool(name="xgp", bufs=1) as xg_pool,
            tc.tile_pool(name="acts", bufs=1) as act_pool,
            tc.tile_pool(name="wstream", bufs=3) as wst_pool,
            tc.tile_pool(name="dstream", bufs=3) as dst_pool,
            tc.tile_pool(name="sstream", bufs=2) as sst_pool,
            tc.tile_pool(name="small", bufs=2) as small_pool,
            tc.tile_pool(name="stage", bufs=3) as stage_pool,
            tc.tile_pool(name="ps", bufs=1, space="PSUM") as ps_pool,
            tc.tile_pool(name="dram", bufs=1, space="DRAM") as dram_pool,
        ):
            # ---------------- resident loads ----------------
            # expert-0 activations first so routed matmuls start before the
            # (large) xt load completes
            xgb = [xg_pool.tile([P, KH * CJ[j]], bf16, name=f"xgb{j}", tag="xgb") for j in range(EPC)]
            nc.sync.dma_start(xgb[0][:], xg_h[0][:])
            xt2 = [res_pool.tile([P, 2 * T], bf16, name=f"xt2_{kk}", tag=f"xt2_{kk}") for kk in range(KH // 2)]
            for kk in range(KH // 2):
                nc.sync.dma_start(
                    xt2[kk][:].rearrange("p (a t) -> p a t", a=2),
                    xt_h[kk * 2 * P:(kk + 1) * 2 * P, :].rearrange("(a p) t -> p a t", p=P))
            xt_t = [xt2[k // 2][:, (k % 2) * T:(k % 2 + 1) * T] for k in range(KH)]
            nc.sync.dma_start(xgb[1][:], xg_h[1][:])
            gwtb = res_pool.tile([P, KH * E], bf16, name="gwtb", tag="gwtb")
            nc.gpsimd.dma_start(gwtb[:], gwtb_h[:])
            maskb = res_pool.tile([P, (T // P) * E], f32, name="maskb", tag="maskb")
            nc.gpsimd.dma_start(maskb[:], maskb_h[:])
            ident = res_pool.tile([P, P], f32, name="ident", tag="ident")
            make_identity(nc, ident[:])
            zbias = res_pool.tile([P, 1], f32, name="zbias", tag="zbias")
            nc.vector.memset(zbias[:], 0.0)

            # combine-weight scratch in HBM: rows 0..T-1 = combine, row T = zeros
            wflat = dram_pool.tile([(T + 1) * E, 1], f32, name="wflat")
            wflat2d = wflat[:].rearrange("(a b) o -> a (b o)", b=E)
            zrow = res_pool.tile([1, E], f32, name="zrow", tag="zrow")
            nc.vector.memset(zrow[:], 0.0)
            nc.gpsimd.dma_start(wflat2d[T:T + 1, :], zrow[:])

            wb = [res_pool.tile([P, CJ[j]], f32, name=f"wb{j}", tag=f"wb{j}") for j in range(EPC)]
            a_t = [[act_pool.tile([P, CJ[j]], bf16, name=f"a{j}_{m}", tag=f"a{j}_{m}") for m in range(MI)]
                   for j in range(EPC)]
            sg_t = [act_pool.tile([P, T], f32, name=f"sg{m}", tag="sgtmp", bufs=2) for m in range(KS)]
            as_t = [act_pool.tile([P, T], bf16, name=f"as{m}", tag=f"as{m}") for m in range(KS)]

            # ---------------- emission sections ----------------
            def emit_gate():
                lgps = ps_pool.tile([E, T], f32, name="lgps", tag="B1", bufs=2)
                for n in range(NT):
                    for k in range(KH):
                        nc.tensor.matmul(
                            lgps[:, n * 512:(n + 1) * 512],
                            lhsT=gwtb[:, k * E:(k + 1) * E],
                            rhs=xt_t[k][:, n * 512:(n + 1) * 512],
                            start=(k == 0), stop=(k == KH - 1),
                        )
                lgsb = res_pool.tile([E, T], f32, name="lgsb", tag="lgsb")
                nc.scalar.copy(lgsb[:], lgps[:])
                for t8 in range(T // P):
                    trps = ps_pool.tile([P, E], f32, name=f"tr{t8}", tag="A1", bufs=4)
                    nc.tensor.transpose(
                        out=trps[:], in_=lgsb[:, t8 * P:(t8 + 1) * P], identity=ident[0:E, 0:E],
                    )
                    sc = small_pool.tile([P, E], f32, name=f"sc{t8}", tag="sc")
                    nc.scalar.activation(sc[:], trps[:], EXP, bias=zbias[:])
                    mskd = small_pool.tile([P, E], f32, name=f"mskd{t8}", tag="mskd")
                    nc.vector.tensor_mul(out=mskd[:], in0=sc[:], in1=maskb[:, t8 * E:(t8 + 1) * E])
                    ssum = small_pool.tile([P, 1], f32, name=f"ssum{t8}", tag="ssum")
                    nc.vector.reduce_sum(ssum[:], mskd[:], axis=X)
                    rsum = small_pool.tile([P, 1], f32, name=f"rsum{t8}", tag="rsum")
                    nc.vector.reciprocal(rsum[:], ssum[:])
                    comb = small_pool.tile([P, E], f32, name=f"comb{t8}", tag="comb")
                    nc.vector.tensor_scalar_mul(comb[:], mskd[:], rsum[:, :1])
                    nc.gpsimd.dma_start(wflat2d[t8 * P:(t8 + 1) * P, :], comb[:])

            def emit_gather(j):
                # per-slot combine weights -> partition-broadcast wb[j]
                off = 0
                while off < CJ[j]:
                    csz = min(P, CJ[j] - off)
                    it = small_pool.tile([P, 1], i32, name=f"it{j}_{off}", tag="it")
                    nc.gpsimd.dma_start(it[:csz], widx_h[COFFJ[j] + off:COFFJ[j] + off + csz, :])
                    wslot = small_pool.tile([P, 1], f32, name=f"ws{j}_{off}", tag="ws")
                    nc.gpsimd.indirect_dma_start(
                        out=wslot[:csz, :], out_offset=None, in_=wflat[:],
                        in_offset=bass.IndirectOffsetOnAxis(ap=it[:csz, :1], axis=0),
                    )
                    wbps = ps_pool.tile([P, P], f32, name=f"wbps{j}_{off}", tag="A1", bufs=4)
                    nc.tensor.transpose(
                        out=wbps[:, :csz],
                        in_=wslot[:csz, :1].to_broadcast([csz, P]),
                        identity=ident[0:csz, 0:csz],
                    )
                    nc.vector.tensor_copy(wb[j][:, off:off + csz], wbps[:, :csz])
                    off += csz

            def emit_upgate(j):
                xg_t = [xgb[j][:, k * CJ[j]:(k + 1) * CJ[j]] for k in range(KH)]
                g_t = [act_pool.tile([P, CJ[j]], f32, name=f"g{j}_{m}", tag="gtmp", bufs=3) for m in range(MI)]
                for m in range(MI):
                    wgub = wst_pool.tile([P, 2 * KH * P], bf16, name=f"wgub{j}_{m}", tag="wblk", bufs=3)
                    nc.sync.dma_start(
                        wgub[:].rearrange("p (a c) -> p a c", a=2),
                        wgu_h[j][m * 2 * P:(m + 1) * 2 * P, :].rearrange("(a p) c -> p a c", p=P))
                    wgb = [wgub[:, k * P:(k + 1) * P] for k in range(KH)]
                    wub = [wgub[:, (KH + k) * P:(KH + k + 1) * P] for k in range(KH)]
                    for (coff, csz) in CHJ[j]:
                        psg = ps_pool.tile([P, csz], f32, name=f"psg{j}_{m}_{coff}", tag="A1", bufs=4)
                        for k in range(KH):
                            nc.tensor.matmul(psg[:], lhsT=wgb[k],
                                             rhs=xg_t[k][:, coff:coff + csz],
                                             start=(k == 0), stop=(k == KH - 1))
                        nc.scalar.activation(g_t[m][:, coff:coff + csz], psg[:], SILU, bias=zbias[:])
                        psu = ps_pool.tile([P, csz], f32, name=f"psu{j}_{m}_{coff}", tag="A1", bufs=4)
                        for k in range(KH):
                            nc.tensor.matmul(psu[:], lhsT=wub[k],
                                             rhs=xg_t[k][:, coff:coff + csz],
                                             start=(k == 0), stop=(k == KH - 1))
                        # a = silu(g) * u straight out of PSUM, rounded to bf16
                        nc.vector.tensor_mul(out=a_t[j][m][:, coff:coff + csz],
                                             in0=g_t[m][:, coff:coff + csz], in1=psu[:])

            def emit_down(j, interleave=None):
                for mg in range(MH // 2):
                    if interleave is not None:
                        interleave(mg)
                    wdb = dst_pool.tile([P, 2 * KI * P], bf16, name=f"wdb{j}_{mg}", tag="wdb", bufs=2)
                    nc.sync.dma_start(
                        wdb[:].rearrange("p (a c) -> p a c", a=2),
                        wd_h[j][mg * 2 * P:(mg + 1) * 2 * P, :].rearrange("(a p) c -> p a c", p=P))
                    for mh in range(2):
                        m = mg * 2 + mh
                        for (coff, csz) in CHJ[j]:
                            psz = ps_pool.tile([P, csz], f32, name=f"psz{j}_{m}_{coff}", tag="A1", bufs=4)
                            for k in range(KI):
                                nc.tensor.matmul(psz[:], lhsT=wdb[:, (mh * KI + k) * P:(mh * KI + k + 1) * P],
                                                 rhs=a_t[j][k][:, coff:coff + csz],
                                                 start=(k == 0), stop=(k == KI - 1))
                            zst = stage_pool.tile([P, csz], bf16, name=f"zst{j}_{m}_{coff}", tag="zst", bufs=2)
                            # combine-weight scaling fused into the eviction
                            nc.vector.tensor_mul(out=zst[:], in0=wb[j][:, coff:coff + csz], in1=psz[:])
                            nc.gpsimd.dma_start(
                                zt_h[m * P:(m + 1) * P, COFFJ[j] + coff:COFFJ[j] + coff + csz], zst[:])

            def emit_shared_ug(mi):
                sgub = sst_pool.tile([P, 2 * KH * P], bf16, name=f"sgub{mi}", tag="ssb", bufs=2)
                nc.sync.dma_start(
                    sgub[:].rearrange("p (a c) -> p a c", a=2),
                    swgu_h[mi * 2 * P:(mi + 1) * 2 * P, :].rearrange("(a p) c -> p a c", p=P))
                psgs = ps_pool.tile([P, T], f32, name=f"psgs{mi}", tag="B1", bufs=2)
                for k in range(KH):
                    for n in range(NT):
                        nc.tensor.matmul(psgs[:, n * 512:(n + 1) * 512],
                                         lhsT=sgub[:, k * P:(k + 1) * P],
                                         rhs=xt_t[k][:, n * 512:(n + 1) * 512],
                                         start=(k == 0), stop=(k == KH - 1))
                nc.scalar.activation(sg_t[mi][:], psgs[:], SILU, bias=zbias[:])
                psus = ps_pool.tile([P, T], f32, name=f"psus{mi}", tag="B1", bufs=2)
                for k in range(KH):
                    for n in range(NT):
                        nc.tensor.matmul(psus[:, n * 512:(n + 1) * 512],
                                         lhsT=sgub[:, (KH + k) * P:(KH + k + 1) * P],
                                         rhs=xt_t[k][:, n * 512:(n + 1) * 512],
                                         start=(k == 0), stop=(k == KH - 1))
                nc.vector.tensor_mul(out=as_t[mi][:], in0=sg_t[mi][:], in1=psus[:])

            def emit_shared_down(ms):
                for mg in ms:
                    sdb = sst_pool.tile([P, 4 * KS * P], bf16, name=f"sdb{mg}", tag="sdb", bufs=2)
                    nc.sync.dma_start(
                        sdb[:].rearrange("p (a c) -> p a c", a=4),
                        swd_h[mg * 4 * P:(mg + 1) * 4 * P, :].rearrange("(a p) c -> p a c", p=P))
                    for mh in range(4):
                        m = mg * 4 + mh
                        psys = ps_pool.tile([P, T], f32, name=f"psys{m}", tag="B1", bufs=2)
                        for ki in range(KS):
                            for n in range(NT):
                                nc.tensor.matmul(psys[:, n * 512:(n + 1) * 512],
                                                 lhsT=sdb[:, (mh * KS + ki) * P:(mh * KS + ki + 1) * P],
                                                 rhs=as_t[ki][:, n * 512:(n + 1) * 512],
                                                 start=(ki == 0), stop=(ki == KS - 1))
                        sstg = stage_pool.tile([P, T], bf16, name=f"sstg{m}", tag="sstage", bufs=2)
                        nc.scalar.copy(sstg[:], psys[:])
                        nc.sync.dma_start(st_h[m * P:(m + 1) * P, :], sstg[:])

            emit_gate()
            emit_upgate(0)
            emit_gather(0)
            emit_shared_ug(0)
            emit_down(0)
            emit_shared_ug(1)
            emit_upgate(1)
            emit_gather(1)
            emit_shared_ug(2)
            emit_down(1)
            emit_shared_down(list(range(MH // 4)))

    nc.compile()
    return nc


def _get_nc(CA, CB):
    if (CA, CB) not in _NC_CACHE:
        _NC_CACHE[(CA, CB)] = _build(CA, CB)
    return _NC_CACHE[(CA, CB)]


def kernel(**inputs):
    global LAST_RESULTS
    from concourse.bass_utils import run_bass_kernel_spmd

    BF16 = _bf16()

    hs = np.asarray(inputs["hidden_states"], dtype=np.float32)
    gate_w = np.asarray(inputs["gate_w"], dtype=np.float32)
    w_gate = np.asarray(inputs["w_gate"], dtype=np.float32)
    w_up = np.asarray(inputs["w_up"], dtype=np.float32)
    w_down = np.asarray(inputs["w_down"], dtype=np.float32)
    sw_gate = np.asarray(inputs["sw_gate"], dtype=np.float32)
    sw_up = np.asarray(inputs["sw_up"], dtype=np.float32)
    sw_down = np.asarray(inputs["sw_down"], dtype=np.float32)

    orig_shape = hs.shape
    x = hs.reshape(-1, H)
    assert x.shape[0] == T

    # ---- host: discrete routing only (top-4 selection + dispatch tables) ----
    logits = x @ gate_w.T
    smax = logits.max(axis=-1, keepdims=True)
    sc = np.exp(logits - smax)
    sc /= sc.sum(axis=-1, keepdims=True)
    order = np.argsort(-sc, axis=-1, kind="stable")[:, :TOPK]
    mask = np.zeros((T, E), dtype=np.float32)
    mask[np.arange(T)[:, None], order] = 1.0
    tok_lists = [np.nonzero(mask[:, e])[0].astype(np.int64) for e in range(E)]

    # balance: pair the i-th most-loaded expert with the i-th least-loaded
    sizes = np.array([len(tk) for tk in tok_lists])
    by_load = np.argsort(-sizes, kind="stable")
    pairs = [(int(by_load[i]), int(by_load[E - 1 - i])) for i in range(NCORES)]
    CA = max(64, int(np.ceil(max(sizes[p[0]] for p in pairs) / 32)) * 32)
    CB = max(64, int(np.ceil(max(sizes[p[1]] for p in pairs) / 32)) * 32)
    CJ = [CA, CB]

    nc = _get_nc(CA, CB)

    xT = np.ascontiguousarray(x.T)
    xTb = xT.astype(BF16)
    # gate weights packed: gwtb[p, k*E + e] = gate_w[e, k*P + p]
    gwtb = np.ascontiguousarray(
        gate_w.T.reshape(KH, P, E).transpose(1, 0, 2).reshape(P, KH * E)).astype(BF16)
    # mask packed: maskb[p, t8*E + e] = mask[t8*P + p, e]
    maskb = np.ascontiguousarray(mask.reshape(T // P, P, E).transpose(1, 0, 2).reshape(P, (T // P) * E))

    # shared slices, zero-padded to 384 and tile-major packed
    def pad_cols(w, newc):
        out = np.zeros((w.shape[0], newc), dtype=np.float32)
        out[:, :w.shape[1]] = w
        return out

    def pad_rows(w, newr):
        out = np.zeros((newr, w.shape[1]), dtype=np.float32)
        out[:w.shape[0], :] = w
        return out

    in_maps = []
    for c in range(NCORES):
        es = pairs[c]
        widx = np.full((CA + CB, 1), ZERO_ROW_FLAT, dtype=np.int32)
        sg_p = _pack_st(pad_cols(sw_gate[:, c * ISS:(c + 1) * ISS], ISSP), KH, KS)
        su_p = _pack_st(pad_cols(sw_up[:, c * ISS:(c + 1) * ISS], ISSP), KH, KS)
        im = {
            "xt": xTb, "gwtb": gwtb, "maskb": maskb, "widx": widx,
            "swgu": _interleave_pairs(sg_p, su_p, KS, KH * P).astype(BF16),
            "swd": _pack_st(pad_rows(sw_down[c * ISS:(c + 1) * ISS, :], ISSP), KS, MH).astype(BF16),
        }
        coff = 0
        for j, e in enumerate(es):
            tk = tok_lists[e]
            C = CJ[j]
            widx[coff:coff + len(tk), 0] = (tk * E + e).astype(np.int32)
            coff += C
            # gathered activations, tile-major: xg[p, k*C + c] = x[tok_c, k*P + p]
            xg = np.zeros((P, KH * C), dtype=BF16)
            g = xTb[:, tk].reshape(KH, P, len(tk)).transpose(1, 0, 2)  # [P, KH, n]
            xg.reshape(P, KH, C)[:, :, :len(tk)] = g
            im[f"xg{j}"] = xg
            wg_p = _pack_st(w_gate[e], KH, MI)
            wu_p = _pack_st(w_up[e], KH, MI)
            im[f"wgu{j}"] = _interleave_pairs(wg_p, wu_p, MI, KH * P).astype(BF16)
            im[f"wd{j}"] = _pack_st(w_down[e], KI, MH).astype(BF16)
        in_maps.append(im)

    trace = bool(int(os.environ.get("BASSMOE_TRACE", "0")))
    kwargs = {}
    if trace:
        kwargs = dict(trace=True, tmpdir=os.environ.get("BASSMOE_TRACE_DIR") or None)
        tcores = os.environ.get("BASSMOE_TRACE_CORES")
        if tcores:
            kwargs["trace_cores"] = [int(x) for x in tcores.split(",")]
            kwargs["stitch_traces"] = False
    res = run_bass_kernel_spmd(nc, in_maps, core_ids=list(range(NCORES)), **kwargs)
    LAST_RESULTS = res

    # ---- host: unshard (scatter-add compact expert outputs + sum partials) ----
    y = np.zeros((T, H), dtype=np.float64)
    st_sum = np.zeros((H, T), dtype=np.float64)
    for c in range(NCORES):
        r = res.results[c]
        st_sum += np.asarray(r["st"], dtype=np.float64)
        coff = 0
        for j, e in enumerate(pairs[c]):
            tk = tok_lists[e]
            zt = np.asarray(r["zt"], dtype=np.float64)
            y[tk] += zt[:, coff:coff + len(tk)].T
            coff += CJ[j]
    y += st_sum.T
    return y.astype(np.float32).reshape(orig_shape)
